# revision 2
# baseline (speedup 1.0000x reference)
"""Trainium2 Bass kernel for nn_MoCA (self-attention + momentum concept attention).

Sharding: pure data parallel — batch dim (B=8) sharded 1 batch per NeuronCore,
weights/concept pool replicated. No collectives.

Per-core algorithm for one batch (C=512, L=64, HW=4096, P=256):
  th|ph = [w_theta; w_phi]*gain @ fm          (one fused fp32r matmul, I=128)
  g     = w_g*gain @ fm, PE-transposed to gT[m, l] with an appended ones
          column so the PV matmul also produces softmax denominators.
  SA    : S^T[m, n] = ph^T th (fp32r), P^T = exp(S^T) (no max subtraction —
          scores are bounded ~|55| and fp32 exp is exact to 1e-5 there),
          attnT|denom = [gT|1]^T @ P^T accumulated in PSUM.
  The exp work (the ACT-engine bottleneck: 16.8M elements) is split between
  the ACT engine (exact spline exp) and the DVE (Schraudolph fast-exp: one
  fused tensor_scalar A*s+B rounded to int16, whose bits ARE bf16 exp(s) to
  ~1.8% rms, zero-mean — softmax-normalization cancels scale errors and PV
  averaging suppresses the rest).
  The torch .view reshape (attn [HW, L] -> lat [L, HW]) is a raw memory
  reinterpret: transpose attnT blocks back to [n, l], normalize by the
  denominator (per-partition scalar), DMA to a DRAM scratch [HW, L] and
  read it back as [L, HW] (same bytes).
  sa_out = w_oT*gain*gamma_sa @ lat + fm      (residual adds split DVE/Pool)
  MoCA  : S2^T = M2 @ sa_out with M2 = phi_c^T W_theta gain precomputed on
          host (enc conv folded in); same exp/PV/normalize path with the
          256-entry concept pool; out = w_oT*gain*gamma_moca @ lat2 + sa_out.
  Inputs are DMA'd straight into fp32r/bf16 SBUF tiles (fp32r is bit-identical
  to fp32; bf16 weights pre-converted on host) — no on-device load casts.
  fm is loaded in per-n-block chunks so phase 2 starts ~3us in, not ~22us.
  One global PSUM pool with two rotating tags ("big" 2-bank x3, "sm"
  1-bank x2) lets consecutive phases share banks without scope barriers.
"""
import sys

if '/opt/trn_rl_repo' not in sys.path:
    sys.path.insert(0, '/opt/trn_rl_repo')

import numpy as np

C, L, H, W, P = 512, 64, 64, 64, 256
HW = H * W
B = 8
N_CORES = 8

# bf16 Schraudolph exp: bits = round(A*s + B) as int16, reinterpret as bf16.
# A = 2^7/ln2; B = 127*2^7 - 0.058*2^7 (zero-mean relative error, rms 1.8%).
A_EXP = 184.6650390625
B_EXP = 16248.576
# pair-steps (of 16 per n-block) whose exp runs on DVE instead of ACT
DVE_EXP_STEPS = (2, 5, 8, 11, 14)

_STATE: dict = {}


def _truncated_store(nc, sb, out_d, sa, NCC, NB):
    import concourse.mybir as mybir
    for cc in range(NCC):
        for nb in range(NB):
            ns = slice(nb * 512, (nb + 1) * 512)
            ob = sb.tile([128, 512], mybir.dt.float32, tag="ob", name="ob", bufs=3)
            nc.vector.tensor_copy(ob[:], sa[cc][:, ns])
            nc.sync.dma_start(out_d[cc * 128:(cc + 1) * 128, ns], ob[:])


def _build_program(reps=1, num_devices=N_CORES, phases=6):
    import concourse.bass as bass
    import concourse.bacc as bacc
    import concourse.mybir as mybir
    from concourse import tile
    from concourse.masks import make_identity

    dt = mybir.dt
    AFT = mybir.ActivationFunctionType
    ALU = mybir.AluOpType
    f32, f32r, bf16, i16 = dt.float32, dt.float32r, dt.bfloat16, dt.int16

    nc = bacc.Bacc("TRN2", target_bir_lowering=False, debug=False,
                   enable_asserts=False, num_devices=num_devices)

    fm_d = nc.dram_tensor("fm", [C, HW], f32r, kind="ExternalInput").ap()
    wthdup_d = nc.dram_tensor("wthdup", [C, 128], f32r, kind="ExternalInput").ap()
    wphdup_d = nc.dram_tensor("wphdup", [C, 128], f32r, kind="ExternalInput").ap()
    wg_d = nc.dram_tensor("wg", [C, L], f32r, kind="ExternalInput").ap()
    wosa_d = nc.dram_tensor("wosa", [L, C], bf16, kind="ExternalInput").ap()
    womo_d = nc.dram_tensor("womo", [L, C], bf16, kind="ExternalInput").ap()
    m2t_d = nc.dram_tensor("m2t", [C, P], f32r, kind="ExternalInput").ap()
    phiT_d = nc.dram_tensor("phiT", [P, L], bf16, kind="ExternalInput").ap()
    out_d = nc.dram_tensor("out", [C, HW], f32, kind="ExternalOutput").ap()

    NB = HW // 512          # 8 n-blocks of 512
    NM = HW // 128          # 32 m-chunks of 128
    NCC = C // 128          # 4 channel chunks

    with tile.TileContext(nc) as tc:
      for _rep in range(reps):
        with tc.tile_pool(name="sb", bufs=1) as sb, \
             tc.tile_pool(name="dram", bufs=1, space="DRAM") as dp:

            sc1 = dp.tile([HW, L], bf16, tag="sc1", name="sc1")
            sc2 = dp.tile([HW, L], bf16, tag="sc2", name="sc2")

            # ---------------- persistent tiles ----------------
            fmr = [sb.tile([128, HW], f32r, tag=f"fmr{i}", name=f"fmr{i}") for i in range(NCC)]

            lat = sb.tile([L, HW], bf16, tag="lat", name="lat")
            lat2 = sb.tile([L, HW], bf16, tag="lat2", name="lat2")
            sa = fmr  # sa_out written in-place over fmr (residual add)
            wthr = [sb.tile([128, 128], f32r, tag=f"wthr{i}", name=f"wthr{i}") for i in range(NCC)]
            wphr = [sb.tile([128, 128], f32r, tag=f"wphr{i}", name=f"wphr{i}") for i in range(NCC)]
            wgr = [sb.tile([128, L], f32r, tag=f"wgr{i}", name=f"wgr{i}") for i in range(NCC)]
            wosab = sb.tile([L, C], bf16, tag="wosab", name="wosab")
            womob = sb.tile([L, C], bf16, tag="womob", name="womob")
            m2r = [sb.tile([128, P], f32r, tag=f"m2r{i}", name=f"m2r{i}") for i in range(NCC)]
            p2w = sb.tile([128, 2 * 65], bf16, tag="p2w", name="p2w")
            id64 = sb.tile([64, 64], f32, tag="id64", name="id64")
            id65 = sb.tile([65, 65], f32, tag="id65", name="id65")

            make_identity(nc, id64[:])
            make_identity(nc, id65[:])
            nc.vector.memset(p2w[:], 1.0)

            # ---------------- phase 1: direct DMA loads (no casts) --------
            # weights on the Pool DGE queue (idle early), fm chunks on SP so
            # the first n-blocks arrive ASAP and phase 2 starts ~3us in.
            for ci in range(NCC):
                nc.gpsimd.dma_start(wthr[ci][:], wthdup_d[ci * 128:(ci + 1) * 128, :])
                nc.gpsimd.dma_start(wphr[ci][:], wphdup_d[ci * 128:(ci + 1) * 128, :])
                nc.gpsimd.dma_start(wgr[ci][:], wg_d[ci * 128:(ci + 1) * 128, :])
            for nb in range(NB):
                ns = slice(nb * 512, (nb + 1) * 512)
                for ci in range(NCC):
                    nc.sync.dma_start(fmr[ci][:, ns], fm_d[ci * 128:(ci + 1) * 128, ns])
            nc.gpsimd.dma_start(wosab[:], wosa_d[:])
            nc.gpsimd.dma_start(womob[:], womo_d[:])
            for ci in range(NCC):
                nc.gpsimd.dma_start(m2r[ci][:], m2t_d[ci * 128:(ci + 1) * 128, :])
            for pc in range(2):
                nc.gpsimd.dma_start(p2w[:, pc * 65:pc * 65 + 64],
                                    phiT_d[pc * 128:(pc + 1) * 128, :])

            # ---------------- phase 2: th and ph and g convs, gT ----------------
            if phases < 2:
                _truncated_store(nc, sb, out_d, sa, NCC, NB)
                continue
            sa_ctx = tc.tile_pool(name="sapool", bufs=1)
            sasb = sa_ctx.__enter__()
            ps_ctx = tc.tile_pool(name="ps", bufs=1, space="PSUM")
            psum = ps_ctx.__enter__()
            if True:
                th = sasb.tile([128, HW], f32r, tag="th", name="th")
                ph = sasb.tile([128, HW], f32r, tag="ph", name="ph")
                gto = sasb.tile([128, NM * 65], bf16, tag="gto", name="gto")
                nc.vector.memset(gto[:], 1.0)
                g_sb = sasb.tile([L, HW], f32, tag="g_sb", name="g_sb")
                for nb in range(NB):
                    ns = slice(nb * 512, (nb + 1) * 512)
                    pst = psum.tile([128, 512], f32, tag="big", name="th_ps", bufs=3, padded_shape=[128, 1024])
                    for ci in range(NCC):
                        nc.tensor.matmul(pst[:], wthr[ci][:], fmr[ci][:, ns],
                                         start=(ci == 0), stop=(ci == NCC - 1))
                    nc.vector.tensor_copy(th[:, ns], pst[:])
                    psp = psum.tile([128, 512], f32, tag="big", name="ph_ps", bufs=3, padded_shape=[128, 1024])
                    for ci in range(NCC):
                        nc.tensor.matmul(psp[:], wphr[ci][:], fmr[ci][:, ns],
                                         start=(ci == 0), stop=(ci == NCC - 1))
                    nc.vector.tensor_copy(ph[:, ns], psp[:])
                    psg = psum.tile([L, 512], f32, tag="big", name="g_ps", bufs=3, padded_shape=[128, 1024])
                    for ci in range(NCC):
                        nc.tensor.matmul(psg[:], wgr[ci][:], fmr[ci][:, ns],
                                         start=(ci == 0), stop=(ci == NCC - 1))
                    nc.vector.tensor_copy(g_sb[:, ns], psg[:])
                for mc in range(NM):
                    tp = psum.tile([128, 64], f32, tag="sm", name="gtp", bufs=2, padded_shape=[128, 512])
                    nc.tensor.transpose(tp[:], g_sb[:, mc * 128:(mc + 1) * 128],
                                        id64[:])
                    nc.vector.tensor_copy(gto[:, mc * 65:mc * 65 + 64], tp[:])

            if phases < 3:
                sa_ctx.__exit__(None, None, None)
                _truncated_store(nc, sb, out_d, sa, NCC, NB)
                continue
            # ---------------- phase 3: self-attention ----------------
            # software-pipelined: PV lags (ST, exp) by LAG pair-steps so the
            # PE queue never stalls waiting on ACT. exp is split ACT/DVE.
            NPAIR = NM // 2          # 16 pair-steps of 2 m-chunks
            LAG = 2
            if True:
                for nb in range(NB):
                    ns = slice(nb * 512, (nb + 1) * 512)
                    pv = psum.tile([65, 512], f32, tag="sm", name="pv", bufs=2, padded_shape=[128, 512])
                    pts = {}
                    for j in range(NPAIR + LAG):
                        if j < NPAIR:
                            st = psum.tile([128, 1024], f32, tag="big", name="st", bufs=3)
                            for h in range(2):
                                mc = 2 * j + h
                                hp = slice(64 * h, 64 * h + 64)
                                nc.tensor.matmul(
                                    st[:, h * 512:(h + 1) * 512],
                                    ph[hp, mc * 128:(mc + 1) * 128],
                                    th[hp, ns], start=True, stop=True,
                                    tile_position=(64 * h, 0))
                            ptt = sasb.tile([128, 1024], bf16, tag="pt",
                                            name="pt", bufs=LAG + 4)
                            if j in DVE_EXP_STEPS:
                                nc.vector.tensor_scalar(
                                    ptt[:].bitcast(i16), st[:],
                                    A_EXP, B_EXP, ALU.mult, ALU.add)
                            else:
                                nc.scalar.activation(ptt[:], st[:], AFT.Exp)
                            pts[j] = ptt
                        if j >= LAG:
                            jj = j - LAG
                            ptt = pts.pop(jj)
                            for h in range(2):
                                mc = 2 * jj + h
                                nc.tensor.matmul(
                                    pv[:], gto[:, mc * 65:(mc + 1) * 65],
                                    ptt[:, h * 512:(h + 1) * 512],
                                    start=(mc == 0), stop=(mc == NM - 1))
                    at = sasb.tile([65, 512], f32, tag="at", name="at", bufs=2)
                    nc.vector.tensor_copy(at[:], pv[:])
                    for k in range(4):
                        tp = psum.tile([128, 65], f32, tag="sm", name="tt", bufs=2, padded_shape=[128, 512])
                        nc.tensor.transpose(tp[:], at[:, k * 128:(k + 1) * 128],
                                            id65[:])
                        rc = sasb.tile([128, 1], f32, tag="rc", name="rc", bufs=2)
                        nc.vector.reciprocal(rc[:], tp[:, 64:65])
                        tb = sasb.tile([128, 64], bf16, tag="tb", name="tb", bufs=4)
                        nc.vector.tensor_scalar_mul(tb[:], tp[:, 0:64], rc[:])
                        n0 = nb * 512 + k * 128
                        nc.sync.dma_start(sc1[n0:n0 + 128, :], tb[:])
                    lat_view = sc1[:].rearrange("(a b) c -> a (b c)", a=L)
                    nc.sync.dma_start(lat[nb * 8:(nb + 1) * 8, :],
                                      lat_view[nb * 8:(nb + 1) * 8, :])
            sa_ctx.__exit__(None, None, None)

            if phases < 4:
                _truncated_store(nc, sb, out_d, sa, NCC, NB)
                continue
            # ------- phases 4+5 merged: per-nb oconv+residual, enc, concept attn -------
            if True:
                moca = {}
                for nb in range(NB + 1):
                    if nb < NB:
                        ns = slice(nb * 512, (nb + 1) * 512)
                        for cc in range(NCC):
                            ps = psum.tile([128, 512], f32, tag="big", name="oc", bufs=3, padded_shape=[128, 1024])
                            nc.tensor.matmul(ps[:],
                                             wosab[:, cc * 128:(cc + 1) * 128],
                                             lat[:, ns], start=True, stop=True)
                            if cc < 2:
                                nc.vector.tensor_add(sa[cc][:, ns], ps[:],
                                                     sa[cc][:, ns])
                            else:
                                tmp = sb.tile([128, 512], f32, tag="rtmp",
                                              name="rtmp", bufs=3)
                                nc.scalar.activation(tmp[:], ps[:], AFT.Copy)
                                nc.gpsimd.tensor_add(sa[cc][:, ns], tmp[:],
                                                     sa[cc][:, ns])
                        s2 = psum.tile([128, 1024], f32, tag="big", name="s2", bufs=3)
                        for pc in range(2):
                            for ci in range(NCC):
                                nc.tensor.matmul(
                                    s2[:, pc * 512:(pc + 1) * 512],
                                    m2r[ci][:, pc * 128:(pc + 1) * 128],
                                    sa[ci][:, ns],
                                    start=(ci == 0), stop=(ci == NCC - 1))
                        p2t = sb.tile([128, 1024], bf16, tag="p2t", name="p2t",
                                      bufs=3)
                        nc.scalar.activation(p2t[:], s2[:], AFT.Exp)
                        moca[nb] = p2t
                    if nb >= 1:
                        nbl = nb - 1
                        p2t = moca.pop(nbl)
                        pv2 = psum.tile([65, 512], f32, tag="sm", name="pv2", bufs=2, padded_shape=[128, 512])
                        for pc in range(2):
                            nc.tensor.matmul(pv2[:], p2w[:, pc * 65:(pc + 1) * 65],
                                             p2t[:, pc * 512:(pc + 1) * 512],
                                             start=(pc == 0), stop=(pc == 1))
                        at2 = sb.tile([65, 512], f32, tag="at2", name="at2",
                                      bufs=2)
                        nc.vector.tensor_copy(at2[:], pv2[:])
                        for k in range(4):
                            tp = psum.tile([128, 65], f32, tag="sm", name="tt2", bufs=2, padded_shape=[128, 512])
                            nc.tensor.transpose(tp[:],
                                                at2[:, k * 128:(k + 1) * 128],
                                                id65[:])
                            rc = sb.tile([128, 1], f32, tag="rc2", name="rc2",
                                         bufs=2)
                            nc.vector.reciprocal(rc[:], tp[:, 64:65])
                            tb = sb.tile([128, 64], bf16, tag="tb2", name="tb2",
                                         bufs=3)
                            nc.vector.tensor_scalar_mul(tb[:], tp[:, 0:64], rc[:])
                            n0 = nbl * 512 + k * 128
                            nc.sync.dma_start(sc2[n0:n0 + 128, :], tb[:])
                        lat2_view = sc2[:].rearrange("(a b) c -> a (b c)", a=L)
                        nc.sync.dma_start(lat2[nbl * 8:(nbl + 1) * 8, :],
                                          lat2_view[nbl * 8:(nbl + 1) * 8, :])

            # ---------------- phase 6: final o-conv + residual ----------------
            if True:
                for nb in range(NB):
                    for cc in range(NCC):
                        ns = slice(nb * 512, (nb + 1) * 512)
                        ps = psum.tile([128, 512], f32, tag="big", name="o2", bufs=3, padded_shape=[128, 1024])
                        nc.tensor.matmul(ps[:], womob[:, cc * 128:(cc + 1) * 128],
                                         lat2[:, ns], start=True, stop=True)
                        ob = sb.tile([128, 512], f32, tag="ob", name="ob", bufs=3)
                        if cc < 2:
                            nc.vector.tensor_add(ob[:], ps[:], sa[cc][:, ns])
                        else:
                            tmp = sb.tile([128, 512], f32, tag="rtmp2",
                                          name="rtmp2", bufs=3)
                            nc.scalar.activation(tmp[:], ps[:], AFT.Copy)
                            nc.gpsimd.tensor_add(ob[:], tmp[:], sa[cc][:, ns])
                        nc.sync.dma_start(out_d[cc * 128:(cc + 1) * 128, ns], ob[:])
            ps_ctx.__exit__(None, None, None)

    nc.compile()
    return nc


def _get_runner(reps=1):
    """Build the Bass program once and return a cached jitted SPMD callable."""
    key = ("runner", reps)
    if key in _STATE:
        return _STATE[key]

    import jax
    import numpy as np
    from jax.experimental.shard_map import shard_map
    from jax.sharding import Mesh, PartitionSpec
    import concourse.mybir as mybir
    from concourse import bass2jax

    nc = _build_program(reps=reps)
    bass2jax.install_neuronx_cc_hook()

    partition_name = (nc.partition_id_tensor.name
                      if nc.partition_id_tensor else None)
    in_names, out_names, out_avals, zero_shapes = [], [], [], []
    for alloc in nc.m.functions[0].allocations:
        if not isinstance(alloc, mybir.MemoryLocationSet):
            continue
        name = alloc.memorylocations[0].name
        if alloc.kind == "ExternalInput":
            if name != partition_name:
                in_names.append(name)
        elif alloc.kind == "ExternalOutput":
            out_names.append(name)
            shape = tuple(alloc.tensor_shape)
            dtype = mybir.dt.np(alloc.dtype)
            out_avals.append(jax.core.ShapedArray(shape, dtype))
            zero_shapes.append((shape, dtype))
    n_params = len(in_names)
    all_in_names = list(in_names) + list(out_names)
    if partition_name is not None:
        all_in_names.append(partition_name)

    def _body(*args):
        operands = list(args)
        if partition_name is not None:
            operands.append(bass2jax.partition_id_tensor())
        outs = bass2jax._bass_exec_p.bind(
            *operands,
            out_avals=tuple(out_avals),
            in_names=tuple(all_in_names),
            out_names=tuple(out_names),
            lowering_input_output_aliases=(),
            sim_require_finite=True,
            sim_require_nnan=True,
            nc=nc,
        )
        return tuple(outs)

    devices = jax.devices()[:N_CORES]
    mesh = Mesh(np.asarray(devices), ("core",))
    n_outs = len(out_names)
    donate = tuple(range(n_params, n_params + n_outs))
    sharded = jax.jit(
        shard_map(_body, mesh=mesh,
                  in_specs=(PartitionSpec("core"),) * (n_params + n_outs),
                  out_specs=(PartitionSpec("core"),) * n_outs,
                  check_rep=False),
        donate_argnums=donate, keep_unused=True)

    runner = {
        "nc": nc, "sharded": sharded, "in_names": in_names,
        "out_names": out_names, "zero_shapes": zero_shapes,
        "n_params": n_params,
    }
    _STATE[key] = runner
    return runner


def _prep_in_maps(feature_map, concepts, w_theta, w_phi, w_g, w_o,
                  gamma_sa, gamma_moca):
    import ml_dtypes
    bf16 = ml_dtypes.bfloat16

    feature_map = np.asarray(feature_map, dtype=np.float32)
    concepts = np.asarray(concepts, dtype=np.float32)
    w_theta = np.asarray(w_theta, dtype=np.float32)
    w_phi = np.asarray(w_phi, dtype=np.float32)
    w_g = np.asarray(w_g, dtype=np.float32)
    w_o = np.asarray(w_o, dtype=np.float32)
    gamma_sa = np.float32(gamma_sa)
    gamma_moca = np.float32(gamma_moca)

    gain = np.float32(1.0 / np.sqrt(C))
    gain_o = np.float32(1.0 / np.sqrt(L))

    wth_t = w_theta.T * gain                                        # [C, L]
    wph_t = w_phi.T * gain
    wthdup = np.ascontiguousarray(np.concatenate([wth_t, wth_t], axis=1))
    wphdup = np.ascontiguousarray(np.concatenate([wph_t, wph_t], axis=1))
    wg_t = np.ascontiguousarray(w_g.T * gain)                       # [C, L]
    wosa = np.ascontiguousarray(
        (w_o.T * (gain_o * gamma_sa)).astype(bf16))                 # [L, C]
    womo = np.ascontiguousarray(
        (w_o.T * (gain_o * gamma_moca)).astype(bf16))               # [L, C]
    m2 = concepts @ (w_theta * gain)            # [P, C]
    m2t = np.ascontiguousarray(m2.T)            # [C, P]
    phiT = np.ascontiguousarray(concepts.astype(bf16))              # [P, L]
    fm_flat = feature_map.reshape(B, C, HW)

    in_maps = []
    for b in range(N_CORES):
        in_maps.append({
            "fm": np.ascontiguousarray(fm_flat[b]),
            "wthdup": wthdup, "wphdup": wphdup, "wg": wg_t,
            "wosa": wosa, "womo": womo, "m2t": m2t, "phiT": phiT,
        })
    return in_maps


def _run(in_maps):
    r = _get_runner()
    n_params = r["n_params"]
    concat_in = [
        np.concatenate([np.asarray(in_maps[c][name])
                        for c in range(N_CORES)], axis=0)
        for name in r["in_names"]
    ]
    concat_zeros = [np.zeros((N_CORES * s[0], *s[1:]), d)
                    for (s, d) in r["zero_shapes"]]
    out_arrs = r["sharded"](*concat_in, *concat_zeros)
    per_core = []
    for c in range(N_CORES):
        per_core.append({
            name: np.asarray(out_arrs[i]).reshape(
                N_CORES, *r["zero_shapes"][i][0])[c]
            for i, name in enumerate(r["out_names"])
        })
    return per_core


def kernel(feature_map, concepts, w_theta, w_phi, w_g, w_o,
           gamma_sa, gamma_moca):
    in_maps = _prep_in_maps(feature_map, concepts, w_theta, w_phi, w_g, w_o,
                            gamma_sa, gamma_moca)
    per_core = _run(in_maps)
    out = np.stack([per_core[b]["out"].reshape(C, H, W)
                    for b in range(B)], axis=0)
    return out.astype(np.float32)


# revision 13
# speedup vs baseline: 30.3977x; 30.3977x over previous
"""Trainium2 Bass kernel for nn_MoCA (self-attention + momentum concept attention).

Sharding: pure data parallel — batch dim (B=8) sharded 1 batch per NeuronCore,
weights/concept pool replicated. No collectives.

Per-core algorithm for one batch (C=512, L=64, HW=4096, P=256):
  th|ph = [w_theta; w_phi]*gain @ fm          (one fused fp32r matmul, I=128)
  g     = w_g*gain @ fm, PE-transposed to gT[m, l] with an appended ones
          column so the PV matmul also produces softmax denominators.
  SA    : S^T[m, n] = ph^T th (fp32r), P^T = exp(S^T) (no max subtraction —
          scores are bounded ~|55| and fp32 exp is exact to 1e-5 there),
          attnT|denom = [gT|1]^T @ P^T accumulated in PSUM.
  The exp work (the ACT-engine bottleneck: 16.8M elements) is split between
  the ACT engine (exact spline exp) and the DVE (Schraudolph fast-exp: one
  fused tensor_scalar A*s+B rounded to int16, whose bits ARE bf16 exp(s) to
  ~1.8% rms, zero-mean — softmax-normalization cancels scale errors and PV
  averaging suppresses the rest).
  The torch .view reshape (attn [HW, L] -> lat [L, HW]) is a raw memory
  reinterpret: transpose attnT blocks back to [n, l], normalize by the
  denominator (per-partition scalar), DMA to a DRAM scratch [HW, L] and
  read it back as [L, HW] (same bytes).
  sa_out = w_oT*gain*gamma_sa @ lat + fm      (residual adds split DVE/Pool)
  MoCA  : S2^T = M2 @ sa_out with M2 = phi_c^T W_theta gain precomputed on
          host (enc conv folded in); same exp/PV/normalize path with the
          256-entry concept pool; out = w_oT*gain*gamma_moca @ lat2 + sa_out.
  Inputs are DMA'd straight into fp32r/bf16 SBUF tiles (fp32r is bit-identical
  to fp32; bf16 weights pre-converted on host) — no on-device load casts.
  fm is loaded in per-n-block chunks so phase 2 starts ~3us in, not ~22us.
  One global PSUM pool with two rotating tags ("big" 2-bank x3, "sm"
  1-bank x2) lets consecutive phases share banks without scope barriers.
"""
import sys

if '/opt/trn_rl_repo' not in sys.path:
    sys.path.insert(0, '/opt/trn_rl_repo')

import numpy as np

C, L, H, W, P = 512, 64, 64, 64, 256
HW = H * W
B = 8
N_CORES = 8

# bf16 Schraudolph exp: bits = round(A*s + B) as int16, reinterpret as bf16.
# A = 2^7/ln2; B = 127*2^7 - 0.058*2^7 (zero-mean relative error, rms 1.8%).
A_EXP = 184.6650390625
B_EXP = 16248.576
# pair-steps (of 16 per n-block) whose exp runs on DVE instead of ACT
DVE_EXP_STEPS = (2, 5, 8, 11, 14)

_STATE: dict = {}


def _truncated_store(nc, sb, out_d, sa, NCC, NB):
    import concourse.mybir as mybir
    for cc in range(NCC):
        for nb in range(NB):
            ns = slice(nb * 512, (nb + 1) * 512)
            ob = sb.tile([128, 512], mybir.dt.float32, tag="ob", name="ob", bufs=3)
            nc.vector.tensor_copy(ob[:], sa[cc][:, ns])
            nc.sync.dma_start(out_d[cc * 128:(cc + 1) * 128, ns], ob[:])


def _build_program(reps=1, num_devices=N_CORES, phases=6):
    import concourse.bass as bass
    import concourse.bacc as bacc
    import concourse.mybir as mybir
    from concourse import tile
    from concourse.masks import make_identity

    dt = mybir.dt
    AFT = mybir.ActivationFunctionType
    ALU = mybir.AluOpType
    f32, f32r, bf16, i16 = dt.float32, dt.float32r, dt.bfloat16, dt.int16

    nc = bacc.Bacc("TRN2", target_bir_lowering=False, debug=False,
                   enable_asserts=False, num_devices=num_devices)

    fm_d = nc.dram_tensor("fm", [C, HW], f32r, kind="ExternalInput").ap()
    wthdup_d = nc.dram_tensor("wthdup", [C, 128], f32r, kind="ExternalInput").ap()
    wphdup_d = nc.dram_tensor("wphdup", [C, 128], f32r, kind="ExternalInput").ap()
    wg_d = nc.dram_tensor("wg", [C, L], f32r, kind="ExternalInput").ap()
    wosa_d = nc.dram_tensor("wosa", [L, C], bf16, kind="ExternalInput").ap()
    womo_d = nc.dram_tensor("womo", [L, C], bf16, kind="ExternalInput").ap()
    m2t_d = nc.dram_tensor("m2t", [C, P], f32r, kind="ExternalInput").ap()
    phiT_d = nc.dram_tensor("phiT", [P, L], bf16, kind="ExternalInput").ap()
    out_d = nc.dram_tensor("out", [C, HW], f32, kind="ExternalOutput").ap()

    NB = HW // 512          # 8 n-blocks of 512
    NM = HW // 128          # 32 m-chunks of 128
    NCC = C // 128          # 4 channel chunks

    with tile.TileContext(nc) as tc:
      for _rep in range(reps):
        with tc.tile_pool(name="sb", bufs=1) as sb, \
             tc.tile_pool(name="dram", bufs=1, space="DRAM") as dp:

            sc1 = dp.tile([HW, L], bf16, tag="sc1", name="sc1")
            sc2 = dp.tile([HW, L], bf16, tag="sc2", name="sc2")

            # ---------------- persistent tiles ----------------
            fmr = [sb.tile([128, HW], f32r, tag=f"fmr{i}", name=f"fmr{i}") for i in range(NCC)]

            # lat/lat2 duplicated into both partition halves (rows 64-127 copy
            # rows 0-63) so phase-4/6 o-convs pair two channel chunks per slot
            lat = sb.tile([128, HW], bf16, tag="lat", name="lat")
            lat2 = sb.tile([128, HW], bf16, tag="lat2", name="lat2")
            sa = fmr  # sa_out written in-place over fmr (residual add)
            wthr = [sb.tile([128, 128], f32r, tag=f"wthr{i}", name=f"wthr{i}") for i in range(NCC)]
            wphr = [sb.tile([128, 128], f32r, tag=f"wphr{i}", name=f"wphr{i}") for i in range(NCC)]
            wgr = [sb.tile([128, L], f32r, tag=f"wgr{i}", name=f"wgr{i}") for i in range(NCC)]
            # w_o weights duplicated into both partition halves so the
            # phase-4/6 o-convs run as row-split tile_position pairs
            wosab = sb.tile([128, C], bf16, tag="wosab", name="wosab")
            womob = sb.tile([128, C], bf16, tag="womob", name="womob")
            m2r = [sb.tile([128, P], f32r, tag=f"m2r{i}", name=f"m2r{i}") for i in range(NCC)]
            p2w = sb.tile([128, 2 * 65], bf16, tag="p2w", name="p2w")
            id64 = sb.tile([64, 64], f32, tag="id64", name="id64")
            id65 = sb.tile([65, 65], f32, tag="id65", name="id65")

            make_identity(nc, id64[:])
            make_identity(nc, id65[:])
            nc.vector.memset(p2w[:], 1.0)

            # ---------------- phase 1: direct DMA loads (no casts) --------
            # weights on the Pool DGE queue (idle early), fm chunks on SP so
            # the first n-blocks arrive ASAP and phase 2 starts ~3us in.
            for ci in range(NCC):
                nc.gpsimd.dma_start(wthr[ci][:], wthdup_d[ci * 128:(ci + 1) * 128, :])
                nc.gpsimd.dma_start(wphr[ci][:], wphdup_d[ci * 128:(ci + 1) * 128, :])
                nc.gpsimd.dma_start(wgr[ci][:], wg_d[ci * 128:(ci + 1) * 128, :])
            for nb in range(NB):
                ns = slice(nb * 512, (nb + 1) * 512)
                for ci in range(NCC):
                    nc.sync.dma_start(fmr[ci][:, ns], fm_d[ci * 128:(ci + 1) * 128, ns])
            nc.gpsimd.dma_start(wosab[0:64, :], wosa_d[:])
            nc.gpsimd.dma_start(wosab[64:128, :], wosa_d[:])
            nc.gpsimd.dma_start(womob[0:64, :], womo_d[:])
            nc.gpsimd.dma_start(womob[64:128, :], womo_d[:])
            for ci in range(NCC):
                nc.gpsimd.dma_start(m2r[ci][:], m2t_d[ci * 128:(ci + 1) * 128, :])
            for pc in range(2):
                nc.gpsimd.dma_start(p2w[:, pc * 65:pc * 65 + 64],
                                    phiT_d[pc * 128:(pc + 1) * 128, :])

            # ---------------- phase 2: th and ph and g convs, gT ----------------
            if phases < 2:
                _truncated_store(nc, sb, out_d, sa, NCC, NB)
                continue
            sa_ctx = tc.tile_pool(name="sapool", bufs=1)
            sasb = sa_ctx.__enter__()
            ps_ctx = tc.tile_pool(name="ps", bufs=1, space="PSUM")
            psum = ps_ctx.__enter__()
            if True:
                th = sasb.tile([128, HW], f32r, tag="th", name="th")
                ph = sasb.tile([128, HW], f32r, tag="ph", name="ph")
                gto = sasb.tile([128, NM * 65], bf16, tag="gto", name="gto")
                nc.vector.memset(gto[:], 1.0)
                g_sb = sasb.tile([L, HW], f32, tag="g_sb", name="g_sb")
                for nb in range(NB):
                    ns = slice(nb * 512, (nb + 1) * 512)
                    pst = psum.tile([128, 512], f32, tag="big", name="th_ps", bufs=3, padded_shape=[128, 1024])
                    for ci in range(NCC):
                        nc.tensor.matmul(pst[:], wthr[ci][:], fmr[ci][:, ns],
                                         start=(ci == 0), stop=(ci == NCC - 1))
                    nc.vector.tensor_copy(th[:, ns], pst[:])
                    psp = psum.tile([128, 512], f32, tag="big", name="ph_ps", bufs=3, padded_shape=[128, 1024])
                    for ci in range(NCC):
                        nc.tensor.matmul(psp[:], wphr[ci][:], fmr[ci][:, ns],
                                         start=(ci == 0), stop=(ci == NCC - 1))
                    nc.vector.tensor_copy(ph[:, ns], psp[:])
                    psg = psum.tile([L, 512], f32, tag="big", name="g_ps", bufs=3, padded_shape=[128, 1024])
                    for ci in range(NCC):
                        nc.tensor.matmul(psg[:], wgr[ci][:], fmr[ci][:, ns],
                                         start=(ci == 0), stop=(ci == NCC - 1))
                    nc.vector.tensor_copy(g_sb[:, ns], psg[:])
                for mc in range(NM):
                    tp = psum.tile([128, 64], f32, tag="sm", name="gtp", bufs=2, padded_shape=[128, 512])
                    nc.tensor.transpose(tp[:], g_sb[:, mc * 128:(mc + 1) * 128],
                                        id64[:])
                    nc.vector.tensor_copy(gto[:, mc * 65:mc * 65 + 64], tp[:])

            if phases < 3:
                sa_ctx.__exit__(None, None, None)
                _truncated_store(nc, sb, out_d, sa, NCC, NB)
                continue
            # ---------------- phase 3: self-attention ----------------
            # software-pipelined: PV lags (ST, exp) by LAG pair-steps so the
            # PE queue never stalls waiting on ACT. exp is split ACT/DVE.
            NPAIR = NM // 2          # 16 pair-steps of 2 m-chunks
            LAG = 2
            if True:
                for nb in range(NB):
                    ns = slice(nb * 512, (nb + 1) * 512)
                    pv = psum.tile([65, 512], f32, tag="sm", name="pv", bufs=2, padded_shape=[128, 512])
                    pts = {}
                    for j in range(NPAIR + LAG):
                        if j < NPAIR:
                            st = psum.tile([128, 1024], f32, tag="big", name="st", bufs=3)
                            for h in range(2):
                                mc = 2 * j + h
                                hp = slice(64 * h, 64 * h + 64)
                                nc.tensor.matmul(
                                    st[:, h * 512:(h + 1) * 512],
                                    ph[hp, mc * 128:(mc + 1) * 128],
                                    th[hp, ns], start=True, stop=True,
                                    tile_position=(64 * h, 0))
                            ptt = sasb.tile([128, 1024], bf16, tag="pt",
                                            name="pt", bufs=LAG + 4)
                            if j in DVE_EXP_STEPS:
                                nc.vector.tensor_scalar(
                                    ptt[:].bitcast(i16), st[:],
                                    A_EXP, B_EXP, ALU.mult, ALU.add)
                            else:
                                nc.scalar.activation(ptt[:], st[:], AFT.Exp)
                            pts[j] = ptt
                        if j >= LAG:
                            jj = j - LAG
                            ptt = pts.pop(jj)
                            for h in range(2):
                                mc = 2 * jj + h
                                nc.tensor.matmul(
                                    pv[:], gto[:, mc * 65:(mc + 1) * 65],
                                    ptt[:, h * 512:(h + 1) * 512],
                                    start=(mc == 0), stop=(mc == NM - 1))
                    at = sasb.tile([65, 512], f32, tag="at", name="at", bufs=2)
                    nc.vector.tensor_copy(at[:], pv[:])
                    for k in range(4):
                        tp = psum.tile([128, 65], f32, tag="sm", name="tt", bufs=2, padded_shape=[128, 512])
                        nc.tensor.transpose(tp[:], at[:, k * 128:(k + 1) * 128],
                                            id65[:])
                        rc = sasb.tile([128, 1], f32, tag="rc", name="rc", bufs=2)
                        nc.vector.reciprocal(rc[:], tp[:, 64:65])
                        tb = sasb.tile([128, 64], bf16, tag="tb", name="tb", bufs=4)
                        nc.vector.tensor_scalar_mul(tb[:], tp[:, 0:64], rc[:])
                        n0 = nb * 512 + k * 128
                        nc.gpsimd.dma_start(sc1[n0:n0 + 128, :], tb[:])
            sa_ctx.__exit__(None, None, None)
            # bulk readback: the .view reshape makes every lat column depend
            # on all of sc1, so phase 4 can't start before the last sc1 write
            # anyway -- one wide 64-partition read beats 8 narrow 8-partition
            # ones (SBUF DMA write parallelism scales with partitions touched)
            lat_view = sc1[:].rearrange("(a b) c -> a (b c)", a=L)
            nc.gpsimd.dma_start(lat[0:64, :], lat_view[:])
            nc.gpsimd.dma_start(lat[64:128, :], lat_view[:])

            if phases < 4:
                _truncated_store(nc, sb, out_d, sa, NCC, NB)
                continue
            # ------- phases 4+5 merged: per-nb oconv+residual, enc, concept attn -------
            if True:
                moca = {}
                for nb in range(NB + 1):
                    if nb < NB:
                        ns = slice(nb * 512, (nb + 1) * 512)
                        for cp in range(2):
                            cca, ccb = cp, cp + 2   # (0,2) then (1,3)
                            pa = psum.tile([128, 512], f32, tag="big", name="oca", bufs=3, padded_shape=[128, 1024])
                            pb = psum.tile([128, 512], f32, tag="big", name="ocb", bufs=3, padded_shape=[128, 1024])
                            nc.tensor.matmul(pa[:],
                                             wosab[0:64, cca * 128:(cca + 1) * 128],
                                             lat[0:64, ns], start=True, stop=True)
                            nc.tensor.matmul(pb[:],
                                             wosab[64:128, ccb * 128:(ccb + 1) * 128],
                                             lat[64:128, ns], start=True, stop=True,
                                             tile_position=(64, 0))
                            nc.vector.tensor_add(sa[cca][:, ns], pa[:],
                                                 sa[cca][:, ns])
                            tmp = sb.tile([128, 512], f32, tag="rtmp",
                                          name="rtmp", bufs=3)
                            nc.scalar.activation(tmp[:], pb[:], AFT.Copy)
                            nc.gpsimd.tensor_add(sa[ccb][:, ns], tmp[:],
                                                 sa[ccb][:, ns])
                        s2 = psum.tile([128, 1024], f32, tag="big", name="s2", bufs=3)
                        for pc in range(2):
                            for ci in range(NCC):
                                nc.tensor.matmul(
                                    s2[:, pc * 512:(pc + 1) * 512],
                                    m2r[ci][:, pc * 128:(pc + 1) * 128],
                                    sa[ci][:, ns],
                                    start=(ci == 0), stop=(ci == NCC - 1))
                        p2t = sb.tile([128, 1024], bf16, tag="p2t", name="p2t",
                                      bufs=3)
                        nc.scalar.activation(p2t[:], s2[:], AFT.Exp)
                        moca[nb] = p2t
                    if nb >= 1:
                        nbl = nb - 1
                        p2t = moca.pop(nbl)
                        pv2 = psum.tile([65, 512], f32, tag="sm", name="pv2", bufs=2, padded_shape=[128, 512])
                        for pc in range(2):
                            nc.tensor.matmul(pv2[:], p2w[:, pc * 65:(pc + 1) * 65],
                                             p2t[:, pc * 512:(pc + 1) * 512],
                                             start=(pc == 0), stop=(pc == 1))
                        at2 = sb.tile([65, 512], f32, tag="at2", name="at2",
                                      bufs=2)
                        nc.vector.tensor_copy(at2[:], pv2[:])
                        for k in range(4):
                            tp = psum.tile([128, 65], f32, tag="sm", name="tt2", bufs=2, padded_shape=[128, 512])
                            nc.tensor.transpose(tp[:],
                                                at2[:, k * 128:(k + 1) * 128],
                                                id65[:])
                            rc = sb.tile([128, 1], f32, tag="rc2", name="rc2",
                                         bufs=2)
                            nc.vector.reciprocal(rc[:], tp[:, 64:65])
                            tb = sb.tile([128, 64], bf16, tag="tb2", name="tb2",
                                         bufs=3)
                            nc.vector.tensor_scalar_mul(tb[:], tp[:, 0:64], rc[:])
                            n0 = nbl * 512 + k * 128
                            nc.gpsimd.dma_start(sc2[n0:n0 + 128, :], tb[:])

            lat2_view = sc2[:].rearrange("(a b) c -> a (b c)", a=L)
            nc.gpsimd.dma_start(lat2[0:64, :], lat2_view[:])
            nc.gpsimd.dma_start(lat2[64:128, :], lat2_view[:])

            # ---------------- phase 6: final o-conv + residual ----------------
            if True:
                for nb in range(NB):
                    ns = slice(nb * 512, (nb + 1) * 512)
                    for cp in range(2):
                        cca, ccb = cp, cp + 2   # (0,2) then (1,3)
                        pa = psum.tile([128, 512], f32, tag="big", name="o2a", bufs=3, padded_shape=[128, 1024])
                        pb = psum.tile([128, 512], f32, tag="big", name="o2b", bufs=3, padded_shape=[128, 1024])
                        nc.tensor.matmul(pa[:], womob[0:64, cca * 128:(cca + 1) * 128],
                                         lat2[0:64, ns], start=True, stop=True)
                        nc.tensor.matmul(pb[:], womob[64:128, ccb * 128:(ccb + 1) * 128],
                                         lat2[64:128, ns], start=True, stop=True,
                                         tile_position=(64, 0))
                        ob = sb.tile([128, 512], f32, tag="ob", name="ob", bufs=3)
                        nc.vector.tensor_add(ob[:], pa[:], sa[cca][:, ns])
                        nc.sync.dma_start(out_d[cca * 128:(cca + 1) * 128, ns], ob[:])
                        tmp = sb.tile([128, 512], f32, tag="rtmp2",
                                      name="rtmp2", bufs=3)
                        nc.scalar.activation(tmp[:], pb[:], AFT.Copy)
                        ob2 = sb.tile([128, 512], f32, tag="ob", name="ob", bufs=3)
                        nc.gpsimd.tensor_add(ob2[:], tmp[:], sa[ccb][:, ns])
                        nc.sync.dma_start(out_d[ccb * 128:(ccb + 1) * 128, ns], ob2[:])
            ps_ctx.__exit__(None, None, None)

    nc.compile()
    return nc


def _get_runner(reps=1):
    """Build the Bass program once and return a cached jitted SPMD callable."""
    key = ("runner", reps)
    if key in _STATE:
        return _STATE[key]

    import jax
    import numpy as np
    from jax.experimental.shard_map import shard_map
    from jax.sharding import Mesh, PartitionSpec
    import concourse.mybir as mybir
    from concourse import bass2jax

    nc = _build_program(reps=reps)
    bass2jax.install_neuronx_cc_hook()

    partition_name = (nc.partition_id_tensor.name
                      if nc.partition_id_tensor else None)
    in_names, out_names, out_avals, zero_shapes = [], [], [], []
    for alloc in nc.m.functions[0].allocations:
        if not isinstance(alloc, mybir.MemoryLocationSet):
            continue
        name = alloc.memorylocations[0].name
        if alloc.kind == "ExternalInput":
            if name != partition_name:
                in_names.append(name)
        elif alloc.kind == "ExternalOutput":
            out_names.append(name)
            shape = tuple(alloc.tensor_shape)
            dtype = mybir.dt.np(alloc.dtype)
            out_avals.append(jax.core.ShapedArray(shape, dtype))
            zero_shapes.append((shape, dtype))
    n_params = len(in_names)
    all_in_names = list(in_names) + list(out_names)
    if partition_name is not None:
        all_in_names.append(partition_name)

    def _body(*args):
        operands = list(args)
        if partition_name is not None:
            operands.append(bass2jax.partition_id_tensor())
        outs = bass2jax._bass_exec_p.bind(
            *operands,
            out_avals=tuple(out_avals),
            in_names=tuple(all_in_names),
            out_names=tuple(out_names),
            lowering_input_output_aliases=(),
            sim_require_finite=True,
            sim_require_nnan=True,
            nc=nc,
        )
        return tuple(outs)

    devices = jax.devices()[:N_CORES]
    mesh = Mesh(np.asarray(devices), ("core",))
    n_outs = len(out_names)
    donate = tuple(range(n_params, n_params + n_outs))
    sharded = jax.jit(
        shard_map(_body, mesh=mesh,
                  in_specs=(PartitionSpec("core"),) * (n_params + n_outs),
                  out_specs=(PartitionSpec("core"),) * n_outs,
                  check_rep=False),
        donate_argnums=donate, keep_unused=True)

    runner = {
        "nc": nc, "sharded": sharded, "in_names": in_names,
        "out_names": out_names, "zero_shapes": zero_shapes,
        "n_params": n_params,
    }
    _STATE[key] = runner
    return runner


def _prep_in_maps(feature_map, concepts, w_theta, w_phi, w_g, w_o,
                  gamma_sa, gamma_moca):
    import ml_dtypes
    bf16 = ml_dtypes.bfloat16

    feature_map = np.asarray(feature_map, dtype=np.float32)
    concepts = np.asarray(concepts, dtype=np.float32)
    w_theta = np.asarray(w_theta, dtype=np.float32)
    w_phi = np.asarray(w_phi, dtype=np.float32)
    w_g = np.asarray(w_g, dtype=np.float32)
    w_o = np.asarray(w_o, dtype=np.float32)
    gamma_sa = np.float32(gamma_sa)
    gamma_moca = np.float32(gamma_moca)

    gain = np.float32(1.0 / np.sqrt(C))
    gain_o = np.float32(1.0 / np.sqrt(L))

    wth_t = w_theta.T * gain                                        # [C, L]
    wph_t = w_phi.T * gain
    wthdup = np.ascontiguousarray(np.concatenate([wth_t, wth_t], axis=1))
    wphdup = np.ascontiguousarray(np.concatenate([wph_t, wph_t], axis=1))
    wg_t = np.ascontiguousarray(w_g.T * gain)                       # [C, L]
    wosa = np.ascontiguousarray(
        (w_o.T * (gain_o * gamma_sa)).astype(bf16))                 # [L, C]
    womo = np.ascontiguousarray(
        (w_o.T * (gain_o * gamma_moca)).astype(bf16))               # [L, C]
    m2 = concepts @ (w_theta * gain)            # [P, C]
    m2t = np.ascontiguousarray(m2.T)            # [C, P]
    phiT = np.ascontiguousarray(concepts.astype(bf16))              # [P, L]
    fm_flat = feature_map.reshape(B, C, HW)

    in_maps = []
    for b in range(N_CORES):
        in_maps.append({
            "fm": np.ascontiguousarray(fm_flat[b]),
            "wthdup": wthdup, "wphdup": wphdup, "wg": wg_t,
            "wosa": wosa, "womo": womo, "m2t": m2t, "phiT": phiT,
        })
    return in_maps


def _run(in_maps):
    r = _get_runner()
    n_params = r["n_params"]
    concat_in = [
        np.concatenate([np.asarray(in_maps[c][name])
                        for c in range(N_CORES)], axis=0)
        for name in r["in_names"]
    ]
    concat_zeros = [np.zeros((N_CORES * s[0], *s[1:]), d)
                    for (s, d) in r["zero_shapes"]]
    out_arrs = r["sharded"](*concat_in, *concat_zeros)
    per_core = []
    for c in range(N_CORES):
        per_core.append({
            name: np.asarray(out_arrs[i]).reshape(
                N_CORES, *r["zero_shapes"][i][0])[c]
            for i, name in enumerate(r["out_names"])
        })
    return per_core


def kernel(feature_map, concepts, w_theta, w_phi, w_g, w_o,
           gamma_sa, gamma_moca):
    in_maps = _prep_in_maps(feature_map, concepts, w_theta, w_phi, w_g, w_o,
                            gamma_sa, gamma_moca)
    per_core = _run(in_maps)
    out = np.stack([per_core[b]["out"].reshape(C, H, W)
                    for b in range(B)], axis=0)
    return out.astype(np.float32)
